# revision 12
# baseline (speedup 1.0000x reference)
"""Trainium2 Bass kernel for nn_SamplingBlock (gnn_message_passing).

Strategy (v2)
-------------
8 cores = (batch b in 0..3) x (vertex half h in 0..1); each core owns 4096
vertices of one batch, fully data-parallel (no collectives).

Host-side weight folding (weights-only algebra, no data computation):
    M_k   = W_sum[:,:,k] @ W_diff          (k = 0..8; [256, 259])
    M_0  += W_center
    bias  = sum_k W_sum[:,:,k] @ b_diff + b_sum + b_center       ([256])
    out[n] = sum_k M_k[:, :256] @ xn_{n,k}
           + [coords of all 9 samplings; 1] @ M_coord-rows + bias
(the coordinate/bias part of all 9 samplings is consolidated into ONE
28-row matmul per 128-point group).

Data layout: the volume is re-laid out on the host as an fp16
"(z,y)-expanded" cell table: element r = z*1024+y*32+x holds the 4 rows
(z,y),(z,y+1),(z+1,y),(z+1,y+1) at that x (clamped at the edge), 256 ch
each.  A trilinear sample's 8 cell corners are then ONE contiguous
4 KiB gather element (elem_size=2048 fp16 elems, elem_step=1024).

Device pipeline per core (Tile framework):
  1. vertex coords -> element row index (int16) + 8 corner weights
  2. ONE dma_gather per 512-point chunk-sampling (8 corners in one elem)
  3. x-lerp on DVE via fused custom op SCALE2 (out = a*s0 + b*s1 with
     per-partition point scalars), fp16 in/out -> 4 partials per group
  4. the 4 partials are transposed AND summed in one shot on the PE:
     4 accumulating identity-matmuls into PSUM (featT = sum_j partial_j^T)
  5. shift matmul (featT stationary) -> neighbour coords -> neighbour
     gather/blend (same machinery)
  6. main matmul: PSUM accumulation of 18 feature matmuls + 1 coord/bias
     matmul -> out [pts, 256]
"""

import os
import sys

import numpy as np

for _p in ("/opt/trn_rl_repo", "/root/.axon_site/_ro/trn_rl_repo"):
    if os.path.isdir(_p) and _p not in sys.path:
        sys.path.insert(0, _p)
        break

import concourse.bacc as bacc
import concourse.bass as bass
import concourse.mybir as mybir
import concourse.tile as tile
from concourse.bass_utils import run_bass_kernel_spmd
from concourse.masks import make_identity

# ---------------------------------------------------------------- constants
B, N, C, NN = 4, 8192, 256, 8
GRID = 32
V = GRID * GRID * GRID            # 32768 elements (one per voxel)
NVC = N // 2                       # vertices per core = 4096
VCHUNK = 512                       # vertices per chunk
NCHUNK = NVC // VCHUNK             # 8
GPC = VCHUNK // 128                # groups (128-pt tiles) per chunk = 4
ES = 2 * 4 * C                     # gather element: 8 corners x 256 ch fp16
STEP = 4 * C                       # element step: one x position = 4 rows
F32 = mybir.dt.float32
F16 = mybir.dt.float16
I16 = mybir.dt.int16
ALU = mybir.AluOpType
MM_DT = F16

# ------------------------------------------------------- custom DVE op SCALE2
_SCALE2 = None


def _register_scale2():
    """out = in0*s0 + in1*s1 (per-partition scalars). Registered once."""
    global _SCALE2
    if _SCALE2 is not None:
        return
    import concourse.dve_ops as dve_ops
    from concourse.dve_spec import C0, C1, Spec, Src0, Src1, lower
    from concourse.dve_uop import DveOpSpec

    for op in dve_ops.OPS:
        if op.name == "SCALE2_GS":
            _SCALE2 = op
            return
    spec = Spec(
        body=Src0 * C0 + Src1 * C1,
        reference=lambda in0, in1, s0, s1, imm2: in0 * s0 + in1 * s1,
    )
    shas = {}
    for ver in ("v3", "v4"):
        tmp = DveOpSpec(name="SCALE2_GS", opcode=0, uops=lower(spec, ver=ver),
                        rd1_en=True)
        shas[ver] = tmp.sha(ver)
    op = dve_ops.DveOp("SCALE2_GS", spec, subdim=False, uops_sha=shas)
    dve_ops.OPS.append(op)
    dve_ops._SUB_OPCODE_FOR_NAME[op.name] = len(dve_ops.OPS) - 1
    dve_ops.CUSTOM_DVE_SPECS[op.name] = spec
    _SCALE2 = op


# ------------------------------------------------------------- device program
def _emit_index_math(nc, sb, coords, npts_free, out_r16, out_w8):
    """coords: [128, npts_free, 3] f32 AP (normalized [-1,1] space, unclipped).
    Writes out_r16 [128, npts_free] int16 element indices and
    out_w8 [128, npts_free, 8] f32 corner weights (order: zy in {00,01,10,11},
    x in {lo,hi} -> w8[..., 2*zy + xh])."""
    S = npts_free
    g = sb.tile([128, S, 3], F32, tag="ixg")
    # g = clip((c+1)*15.5, 0, 31)
    nc.vector.tensor_scalar(g[:], coords, 15.5, 15.5, op0=ALU.mult, op1=ALU.add)
    nc.vector.tensor_scalar(g[:], g[:], float(GRID - 1), 0.0, op0=ALU.min,
                            op1=ALU.max)
    # floor(g) robust to HW f32->int rounding mode: q = int(g); q -= (g < q)
    qi = sb.tile([128, S, 3], mybir.dt.int32, tag="ixq")
    nc.vector.tensor_copy(qi[:], g[:])
    i0 = sb.tile([128, S, 3], F32, tag="ixi")
    nc.vector.tensor_copy(i0[:], qi[:])
    frc = sb.tile([128, S, 3], F32, tag="ixf")
    nc.vector.tensor_tensor(frc[:], g[:], i0[:], op=ALU.subtract)  # g - q
    msk = sb.tile([128, S, 3], F32, tag="ixm")
    nc.vector.tensor_scalar(msk[:], frc[:], 0.0, None, op0=ALU.is_lt)
    nc.vector.tensor_tensor(i0[:], i0[:], msk[:], op=ALU.subtract)
    nc.vector.tensor_tensor(frc[:], g[:], i0[:], op=ALU.subtract)
    # r = z*1024 + y*32 + x   (exact in f32)
    r = sb.tile([128, S], F32, tag="ixr")
    nc.vector.tensor_scalar(r[:], i0[:, :, 2:3].squeeze(2), 1024.0, None,
                            op0=ALU.mult)
    t = sb.tile([128, S], F32, tag="ixt")
    nc.vector.tensor_scalar(t[:], i0[:, :, 1:2].squeeze(2), 32.0, None,
                            op0=ALU.mult)
    nc.vector.tensor_tensor(r[:], r[:], t[:], op=ALU.add)
    nc.vector.tensor_tensor(r[:], r[:], i0[:, :, 0:1].squeeze(2), op=ALU.add)
    nc.vector.tensor_copy(out_r16, r[:])
    # weights: a=fx, b=fy, c=fz
    inv = sb.tile([128, S, 3], F32, tag="ixv")   # 1-f
    nc.vector.tensor_scalar(inv[:], frc[:], -1.0, 1.0, op0=ALU.mult, op1=ALU.add)
    wzy = sb.tile([128, S, 4], F32, tag="ixw")
    # zy order: 00:(1-fy)(1-fz) 01:fy(1-fz) 10:(1-fy)fz 11:fy*fz
    yz = [(inv, inv), (frc, inv), (inv, frc), (frc, frc)]
    for k, (ysrc, zsrc) in enumerate(yz):
        nc.vector.tensor_tensor(
            wzy[:, :, k : k + 1].squeeze(2),
            ysrc[:, :, 1:2].squeeze(2),
            zsrc[:, :, 2:3].squeeze(2),
            op=ALU.mult,
        )
    for k in range(4):
        nc.vector.tensor_tensor(
            out_w8[:, :, 2 * k : 2 * k + 1].squeeze(2),
            wzy[:, :, k : k + 1].squeeze(2),
            inv[:, :, 0:1].squeeze(2), op=ALU.mult)
        nc.vector.tensor_tensor(
            out_w8[:, :, 2 * k + 1 : 2 * k + 2].squeeze(2),
            wzy[:, :, k : k + 1].squeeze(2),
            frc[:, :, 0:1].squeeze(2), op=ALU.mult)


def _col(ap3, g, j):
    """[128, G, J] tile -> [128, 1] scalar AP at (g, j)."""
    return ap3[:, g : g + 1, j : j + 1].squeeze(2)


def build_program(nvc=NVC):
    _register_scale2()
    nchunk = nvc // VCHUNK
    nc = bacc.Bacc("TRN2", target_bir_lowering=False, debug=False)

    verts_d = nc.dram_tensor("verts", [nvc, 3], F32, kind="ExternalInput")
    table_d = nc.dram_tensor("table", [(V + 1) * STEP], F16,
                             kind="ExternalInput")
    msum_a_d = nc.dram_tensor("msum_a", [128, 9, C], MM_DT, kind="ExternalInput")
    msum_b_d = nc.dram_tensor("msum_b", [128, 9, C], MM_DT, kind="ExternalInput")
    msum_cc_d = nc.dram_tensor("msum_cc", [28, C], MM_DT, kind="ExternalInput")
    wsh_a_d = nc.dram_tensor("wsh_a", [128, 3 * NN], MM_DT, kind="ExternalInput")
    wsh_b_d = nc.dram_tensor("wsh_b", [128, 3 * NN], MM_DT, kind="ExternalInput")
    bshr_d = nc.dram_tensor("bshr", [128, 3 * NN], F32, kind="ExternalInput")
    rep16_d = nc.dram_tensor("rep16", [16, 128], F32, kind="ExternalInput")
    out_d = nc.dram_tensor("out", [nvc, C], F32, kind="ExternalOutput")

    tbl_ap = bass.AP(table_d, 0, [[STEP, V], [1, ES]])

    with tile.TileContext(nc) as tc:
        with (
            tc.tile_pool(name="const", bufs=1) as cst,
            tc.tile_pool(name="wts", bufs=1) as wp,
            tc.tile_pool(name="ix", bufs=2) as ixp,
            tc.tile_pool(name="gat", bufs=2) as gp,
            tc.tile_pool(name="blend", bufs=2) as bp,
            tc.tile_pool(name="feat", bufs=3) as fp,
            tc.tile_pool(name="misc", bufs=2) as mp,
            tc.tile_pool(name="dram", bufs=2, space="DRAM") as dp,
            tc.tile_pool(name="pso", bufs=1, space="PSUM") as pso,
            tc.tile_pool(name="pst", bufs=1, space="PSUM") as pst,
            tc.tile_pool(name="pss", bufs=1, space="PSUM") as pss,
        ):
            ident32 = cst.tile([128, 128], F32)
            make_identity(nc, ident32[:])
            ident16 = cst.tile([128, 128], F16)
            make_identity(nc, ident16[:])
            msum_a = cst.tile([128, 9, C], MM_DT)
            msum_b = cst.tile([128, 9, C], MM_DT)
            msum_cc = cst.tile([28, C], MM_DT)
            wsh_a = cst.tile([128, 3 * NN], MM_DT)
            wsh_b = cst.tile([128, 3 * NN], MM_DT)
            bshr = cst.tile([128, 3 * NN], F32)
            rep16 = cst.tile([16, 128], F32)
            nc.sync.dma_start(msum_a[:], msum_a_d[:])
            nc.sync.dma_start(msum_b[:], msum_b_d[:])
            nc.sync.dma_start(msum_cc[:], msum_cc_d[:])
            nc.sync.dma_start(wsh_a[:], wsh_a_d[:])
            nc.sync.dma_start(wsh_b[:], wsh_b_d[:])
            nc.sync.dma_start(bshr[:], bshr_d[:])
            nc.sync.dma_start(rep16[:], rep16_d[:])

            verts = cst.tile([128, nvc // 128, 3], F32)
            nc.sync.dma_start(
                verts[:], verts_d[:].rearrange("(vt p) c -> p vt c", p=128))

            # ---- whole-core center index math ----
            r16c = wp.tile([128, nvc // 128], I16)
            w8c = wp.tile([128, nvc // 128, 8], F32)
            _emit_index_math(nc, wp, verts[:], nvc // 128, r16c[:], w8c[:])
            scr_c = dp.tile([nvc], I16)
            nc.sync.dma_start(
                scr_c[:].rearrange("(vt p) -> p vt", p=128), r16c[:])

            def rep_idx(f16t, width, tag):
                """[16, width] f32 wrapped idx -> [128, width] i16 replicated."""
                pr = pss.tile([128, NN * VCHUNK // 16], F32, space="PSUM",
                              tag="rep", name="pr")
                nc.tensor.matmul(pr[:, 0:width], rep16[:], f16t,
                                 start=True, stop=True)
                it = ixp.tile([128, width], I16, tag=tag)
                nc.vector.tensor_copy(it[:], pr[:, 0:width])
                return it

            def load_idx_c(scr_ap):
                t16 = ixp.tile([16, VCHUNK // 16], I16, tag="idx16")
                nc.sync.dma_start(
                    t16[:], scr_ap.rearrange("(m q) -> q m", q=16))
                f16t = ixp.tile([16, VCHUNK // 16], F32, tag="idxf")
                nc.vector.tensor_copy(f16t[:], t16[:])
                return rep_idx(f16t[:], VCHUNK // 16, "idxc")

            def gather512(idx_ap, tag):
                gt = gp.tile([128, GPC, ES], F16, tag=tag)
                nc.gpsimd.dma_gather(
                    gt[:], tbl_ap, idx_ap, VCHUNK, VCHUNK, ES, elem_step=STEP)
                return gt

            def blend_featT(gt, g, wap, tag):
                """One 128-pt group: gathered elem -> featT in PSUM.
                Returns (fts0, fts1) fp16 [128ch, 128pt] SBUF tiles."""
                parts = []
                for u in range(4):
                    pu = bp.tile([128, C], F16, tag=f"p{u}")
                    nc.vector._custom_dve(
                        _SCALE2, out=pu[:],
                        in0=gt[:, g, u * C : (u + 1) * C],
                        in1=gt[:, g, 4 * C + u * C : 4 * C + (u + 1) * C],
                        s0=_col(wap, 0, 2 * u), s1=_col(wap, 0, 2 * u + 1))
                    parts.append(pu)
                fts = []
                for h in range(2):
                    ftp = pst.tile([128, 128], F32, space="PSUM", tag=f"ftp{h}",
                                   name=f"ftp{h}")
                    for u in range(4):
                        nc.tensor.matmul(
                            ftp[:], parts[u][:, h * 128 : (h + 1) * 128],
                            ident16[:], start=(u == 0), stop=(u == 3))
                    ft = fp.tile([128, 128], F16, tag=f"ft{h}{tag}",
                                 name=f"ft{h}")
                    nc.scalar.copy(ft[:], ftp[:])
                    fts.append(ft)
                return fts

            for vc in range(nchunk):
                # ================= centers =================
                idx_c = load_idx_c(scr_c[vc * VCHUNK : (vc + 1) * VCHUNK])
                gts = gather512(idx_c[:], "gc")
                out_ps = [
                    pso.tile([128, C], F32, space="PSUM", tag=f"o{g}",
                             name=f"ops{g}")
                    for g in range(GPC)
                ]
                ncoord = mp.tile([128, GPC, NN, 3], F32, tag="ncrd")
                for g in range(GPC):
                    vt = vc * GPC + g
                    fts = blend_featT(gts, g, w8c[:, vt : vt + 1, :], "c")
                    # shift matmul -> [128 pts, 24] (bank shared with ctp)
                    shct = pss.tile([128, 128], F32, space="PSUM", tag="shct",
                                    name="shct")
                    nc.tensor.matmul(shct[:, 0 : 3 * NN], fts[0][:], wsh_a[:],
                                     start=True, stop=False)
                    nc.tensor.matmul(shct[:, 0 : 3 * NN], fts[1][:], wsh_b[:],
                                     start=False, stop=True)
                    ssb = mp.tile([128, 3 * NN], F32, tag="ssb")
                    nc.scalar.copy(ssb[:], shct[:, 0 : 3 * NN])
                    # main matmul k=0
                    nc.tensor.matmul(out_ps[g][:], fts[0][:], msum_a[:, 0, :],
                                     start=True, stop=False)
                    nc.tensor.matmul(out_ps[g][:], fts[1][:], msum_b[:, 0, :],
                                     start=False, stop=False)
                    # neighbour coords: verts + shift + b_shift  [128, NN, 3]
                    nc.vector.tensor_tensor(
                        ncoord[:, g, :, :].rearrange("p nn c -> p (nn c)"),
                        ssb[:], bshr[:], op=ALU.add)
                    nc.vector.tensor_tensor(
                        ncoord[:, g, :, :],
                        ncoord[:, g, :, :],
                        verts[:, vt : vt + 1, :].to_broadcast([128, NN, 3]),
                        op=ALU.add)
                    # coords tile [128, 28]: center(3) + neighbours(24) + ones
                    ctile = mp.tile([128, 28], F32, tag="ctl")
                    nc.vector.tensor_copy(ctile[:, 0:3], verts[:, vt, :])
                    nc.vector.tensor_copy(
                        ctile[:, 3:27],
                        ncoord[:, g, :, :].rearrange("p nn c -> p (nn c)"))
                    nc.vector.memset(ctile[:, 27:28], 1.0)
                    ctp = pss.tile([128, 128], F32, space="PSUM", tag="shct",
                                   name="ctp")
                    nc.tensor.transpose(ctp[:28, :], ctile[:], ident32[:])
                    ctn = mp.tile([28, 128], F16, tag="ctn")
                    nc.scalar.copy(ctn[:], ctp[:28, :])
                    # coords + bias contribution for all 9 samplings
                    nc.tensor.matmul(out_ps[g][:], ctn[:], msum_cc[:],
                                     start=False, stop=False)
                # ============ neighbour index math (whole chunk) ============
                r16n = ixp.tile([128, GPC * NN], I16, tag="r16n")
                w8n = ixp.tile([128, GPC * NN, 8], F32, tag="w8n")
                _emit_index_math(
                    nc, ixp,
                    ncoord[:].rearrange("p g nn c -> p (g nn) c"),
                    GPC * NN, r16n[:], w8n[:])
                scr_n = dp.tile([GPC * NN * 128], I16, tag="scrn")
                # scr_n order (nn, g, p); one 2D DMA per nn
                r16n_v = r16n[:].rearrange("p (g nn) -> p g nn", nn=NN)
                for nn_i in range(NN):
                    nc.sync.dma_start(
                        scr_n[nn_i * VCHUNK : (nn_i + 1) * VCHUNK].rearrange(
                            "(g p) -> p g", p=128),
                        r16n_v[:, :, nn_i])
                # ================= neighbours =================
                t16n = ixp.tile([16, NN * VCHUNK // 16], I16, tag="t16n")
                nc.sync.dma_start(
                    t16n[:],
                    scr_n[:].rearrange("(nn m q) -> q (nn m)", q=16, nn=NN))
                f16n = ixp.tile([16, NN * VCHUNK // 16], F32, tag="f16n")
                nc.vector.tensor_copy(f16n[:], t16n[:])
                idx_n = rep_idx(f16n[:], NN * VCHUNK // 16, "idxn")
                W = VCHUNK // 16
                for nn_i in range(NN):
                    gtn = gather512(idx_n[:, nn_i * W : (nn_i + 1) * W], "gn")
                    for g in range(GPC):
                        fts = blend_featT(
                            gtn, g, w8n[:, g * NN + nn_i : g * NN + nn_i + 1, :],
                            "n")
                        nc.tensor.matmul(
                            out_ps[g][:], fts[0][:], msum_a[:, nn_i + 1, :],
                            start=False, stop=False)
                        nc.tensor.matmul(
                            out_ps[g][:], fts[1][:], msum_b[:, nn_i + 1, :],
                            start=False,
                            stop=(nn_i == NN - 1))
                # ================= epilogue =================
                osb = mp.tile([128, GPC, C], F32, tag="osb")
                for g in range(GPC):
                    nc.scalar.copy(osb[:, g, :], out_ps[g][:])
                lo = vc * VCHUNK
                nc.sync.dma_start(
                    out_d[lo : lo + VCHUNK, :].rearrange(
                        "(g p) c -> p g c", p=128),
                    osb[:])

    nc.compile()
    return nc


# --------------------------------------------------------------- host wrapper
_CACHED = {}


def _host_prep(x, W_shift, b_shift, W_diff, b_diff, W_center, b_center,
               W_sum, b_sum):
    # (z,y)-expanded fp16 cell table per batch: element r = z*1024+y*32+x
    # holds rows (z,y),(z,y+1),(z+1,y),(z+1,y+1) at x, 256 ch each.
    xt = np.ascontiguousarray(
        np.transpose(x.reshape(B, C, GRID, GRID, GRID), (0, 2, 3, 4, 1))
    ).astype(np.float16)                                       # [B,z,y,x,C]
    P = np.pad(xt, ((0, 0), (0, 1), (0, 1), (0, 0), (0, 0)), mode="edge")
    table = np.zeros((B, V + 1, STEP), np.float16)
    tv = table[:, :V].reshape(B, GRID, GRID, GRID, 4, C)
    tv[..., 0, :] = P[:, :GRID, :GRID]
    tv[..., 1, :] = P[:, :GRID, 1:]
    tv[..., 2, :] = P[:, 1:, :GRID]
    tv[..., 3, :] = P[:, 1:, 1:]
    table = table.reshape(B, (V + 1) * STEP)

    M = np.einsum("ock,cd->okd", W_sum.astype(np.float64),
                  W_diff.astype(np.float64))                  # [256, 9, 259]
    M = np.transpose(M, (1, 0, 2))                            # [9, 256, 259]
    M = M.copy()
    M[0] += W_center.astype(np.float64)
    bias = (W_sum.astype(np.float64).sum(-1) @ b_diff.astype(np.float64)
            + b_sum + b_center)                               # [256]
    # feature part, split by input-channel half
    msum_a = np.ascontiguousarray(
        np.transpose(M[:, :, 0:128], (2, 0, 1))).astype(np.float16)
    msum_b = np.ascontiguousarray(
        np.transpose(M[:, :, 128:256], (2, 0, 1))).astype(np.float16)
    # consolidated coordinate/bias rows: [28, 256]
    msum_cc = np.zeros((28, C), np.float16)
    msum_cc[0:3] = M[0][:, 256:259].T.astype(np.float16)
    for i in range(NN):
        msum_cc[3 + 3 * i : 6 + 3 * i] = \
            M[i + 1][:, 256:259].T.astype(np.float16)
    msum_cc[27] = bias.astype(np.float16)

    wsh = W_shift.T.astype(np.float16)                        # [256, 24]
    bshr = np.tile(b_shift.astype(np.float32), (128, 1))      # [128, 24]
    return table, msum_a, msum_b, msum_cc, wsh, bshr


def kernel(x, vertices, W_shift, b_shift, W_diff, b_diff, W_center, b_center,
           W_sum, b_sum):
    if "nc" not in _CACHED:
        _CACHED["nc"] = build_program()
    nc = _CACHED["nc"]

    table, msum_a, msum_b, msum_cc, wsh, bshr = _host_prep(
        x, W_shift, b_shift, W_diff, b_diff, W_center, b_center, W_sum, b_sum)

    in_maps = []
    for core in range(8):
        b, h = divmod(core, 2)
        in_maps.append({
            "verts": np.ascontiguousarray(
                vertices[b, h * NVC : (h + 1) * NVC]).astype(np.float32),
            "table": table[b],
            "msum_a": msum_a, "msum_b": msum_b, "msum_cc": msum_cc,
            "wsh_a": np.ascontiguousarray(wsh[0:128]),
            "wsh_b": np.ascontiguousarray(wsh[128:256]),
            "bshr": bshr,
            "rep16": np.tile(np.eye(16, dtype=np.float32), 8),
        })

    res = run_bass_kernel_spmd(nc, in_maps, core_ids=list(range(8)))
    out = np.empty((B, N, C), np.float32)
    for core in range(8):
        b, h = divmod(core, 2)
        out[b, h * NVC : (h + 1) * NVC] = res.results[core]["out"]
    return out
